# revision 43
# baseline (speedup 1.0000x reference)
"""Bass/Tile kernel builder for sharded LlamaAttention on TRN2.

Per-core problem (8 cores = 2 batch groups x 4 head groups):
  inputs (per core, bf16 unless noted):
    xT    [D=2048, S=2048]  hidden_states[b].T
    wqT   [D=2048, 512]     wq rows for this core's 4 q heads, transposed
    wkT   [D=2048, 128]     wk rows for this core's kv head, transposed
    wvT   [D=2048, 128]
    woT   [512, D=2048]     wo cols for this core's heads, transposed
    cosT  [128, S=2048]     cos[b].T
    sinT  [128, S=2048]
  output:
    out   [S=2048, D=2048]  f32 partial output (this head group's contribution)

All matmuls run in bf16 (fp32 PSUM accumulation); fp32 matmul mode on TRN2
throttles the PE to half rate.  Layouts: qT/kT kept transposed [HD, S] so the
softmax weights p[k, q] feed the AV matmul as the moving operand with no
on-chip transposes; V natural [S, HD].  The softmax denominator accumulates
on the tensor engine (ones-column matmul into PSUM) instead of DVE adds, and
causal diagonal tiles only compute the live q-range.
"""

import sys
sys.path.insert(0, '/opt/trn_rl_repo')

from contextlib import ExitStack

import concourse.bass as bass
import concourse.tile as tile
import concourse.mybir as mybir
from concourse import bacc
from concourse.alu_op_type import AluOpType
from concourse.masks import make_identity

F32 = mybir.dt.float32
BF16 = mybir.dt.bfloat16

S = 2048
D = 2048
HD = 128
NQH = 4            # q heads per core
SC = 512           # projection S-chunk width / attention q-chunk width
NSC = S // SC      # 4
ND = D // 128      # 16 contraction chunks
NQT = S // 128     # 16 q tiles
SCALE = HD ** -0.5
MASK_NEG = -1e12


def build_kernel(bf16_tr=True, fast_recip=True, gps_bcast=True, trim=True,
                 dn_mm=True, psum_dma=False, lookahead=3, o_bufs=2, debug_dump=False):
    nc = bacc.Bacc(trn_type="TRN2", target_bir_lowering=False, debug=False,
                   num_devices=1)
    xT = nc.dram_tensor("xT", [D, S], BF16, kind="ExternalInput").ap()
    wqT = nc.dram_tensor("wqT", [D, NQH * HD], BF16, kind="ExternalInput").ap()
    wkT = nc.dram_tensor("wkT", [D, HD], BF16, kind="ExternalInput").ap()
    wvT = nc.dram_tensor("wvT", [D, HD], BF16, kind="ExternalInput").ap()
    woT = nc.dram_tensor("woT", [NQH * HD, D], BF16, kind="ExternalInput").ap()
    cosT = nc.dram_tensor("cosT", [HD, S], BF16, kind="ExternalInput").ap()
    sinT = nc.dram_tensor("sinT", [HD, S], BF16, kind="ExternalInput").ap()
    out = nc.dram_tensor("out", [S, D], F32, kind="ExternalOutput").ap()

    with tile.TileContext(nc) as tc, ExitStack() as top:
        # ---------- resident pools ----------
        consts = top.enter_context(tc.tile_pool(name="consts", bufs=1))
        qkpool = top.enter_context(tc.tile_pool(name="qk", bufs=1))
        vpool = top.enter_context(tc.tile_pool(name="vnat", bufs=1))
        wopool = top.enter_context(tc.tile_pool(name="wo", bufs=1))

        identf = consts.tile([128, 128], F32, tag="identf")
        make_identity(nc, identf[:])
        TRDT = BF16 if bf16_tr else F32
        if bf16_tr:
            ident = consts.tile([128, 128], BF16, tag="ident")
            nc.scalar.copy(ident[:], identf[:])
        else:
            ident = identf
        ones_f32 = consts.tile([128, 128], F32, tag="ones_f32")
        nc.gpsimd.memset(ones_f32[:], 1.0)
        F32R = mybir.dt.float32r
        ones_col = consts.tile([128, 1], BF16, tag="ones_col")
        nc.scalar.copy(ones_col[:], ones_f32[:, 0:1])
        ones_col_r = consts.tile([128, 1], F32R, tag="ones_col_r")
        nc.scalar.copy(ones_col_r[:], ones_f32[:, 0:1])
        # mext[r, cc] = 0 if cc >= 384 + r else MASK_NEG ; [128, 512]
        # band slice mext[:, 384:512] is the triangular diagonal mask
        mext = consts.tile([128, 512], F32, tag="mext")
        nc.gpsimd.memset(mext[:], 0.0)
        nc.gpsimd.affine_select(
            out=mext[:], in_=mext[:],
            compare_op=AluOpType.is_ge,
            fill=MASK_NEG,
            base=-384,
            pattern=[[1, 512]],
            channel_multiplier=-1,
        )

        # q/k transposed+roped [HD, S]; v natural [S, HD] as 16 [128,128]
        qT_sb = [qkpool.tile([128, S], BF16, tag=f"qT{h}", name=f"qT{h}") for h in range(NQH)]
        kT_sb = qkpool.tile([128, S], BF16, tag="kT", name="kT")
        v_sb = [vpool.tile([128, HD], BF16, tag=f"v{i}", name=f"v{i}") for i in range(NQT)]
        woT_sb = [wopool.tile([128, D], BF16, tag=f"wo{h}", name=f"wo{h}") for h in range(NQH)]

        # ================= Stage A: projections + rope =================
        with ExitStack() as ctxa:
            wq_pool = ctxa.enter_context(tc.tile_pool(name="wq", bufs=ND))
            wkv_pool = ctxa.enter_context(tc.tile_pool(name="wkv", bufs=2))
            cs_pool = ctxa.enter_context(tc.tile_pool(name="cs", bufs=2))
            xt_pool = ctxa.enter_context(tc.tile_pool(name="xt", bufs=18))
            rs_pool = ctxa.enter_context(tc.tile_pool(name="ropes", bufs=2))
            pj_ps = ctxa.enter_context(
                tc.tile_pool(name="pj_ps", bufs=6, space="PSUM"))
            tr_ps = ctxa.enter_context(
                tc.tile_pool(name="tr_ps", bufs=2, space="PSUM"))

            wq_t = [wq_pool.tile([128, NQH * HD], BF16, tag="wq", name="wqt") for _ in range(ND)]
            # wk/wv: [D,HD] rearranged into [128, ND*HD] (chunk d at cols d*HD)
            wk_t = wkv_pool.tile([128, ND * HD], BF16, tag="wkv")
            wv_t = wkv_pool.tile([128, ND * HD], BF16, tag="wkv")
            cos_t = cs_pool.tile([128, S], BF16, tag="cs")
            sin_t = cs_pool.tile([128, S], BF16, tag="cs")
            nc.sync.dma_start(
                wk_t[:].rearrange("p (d h) -> p d h", h=HD),
                wkT.rearrange("(d p) h -> p d h", p=128))
            nc.sync.dma_start(
                wv_t[:].rearrange("p (d h) -> p d h", h=HD),
                wvT.rearrange("(d p) h -> p d h", p=128))
            xp_t = [None] * ND

            def rope(dst_slice, x_ps, c_sl, s_sl, t1, t2, eng=None):
                # dst = x*c + rot_half(x)*s   (all [128, SC], x in PSUM)
                eng = eng or nc.vector
                eng.tensor_tensor(t1[:], x_ps[:], c_sl, AluOpType.mult)
                # t2[0:64] = -x[64:128]*s[0:64]
                eng.scalar_tensor_tensor(
                    t2[0:64, :], x_ps[64:128, :], -1.0, s_sl[0:64, :],
                    op0=AluOpType.mult, op1=AluOpType.mult)
                # t2[64:128] = x[0:64]*s[64:128]
                eng.tensor_tensor(
                    t2[64:128, :], x_ps[0:64, :], s_sl[64:128, :], AluOpType.mult)
                eng.tensor_tensor(dst_slice, t1[:], t2[:], AluOpType.add)

            for sc in range(NSC):
                ssl = bass.ts(sc, SC)
                q_ps = [pj_ps.tile([128, SC], F32, tag="pj", name="qps") for _ in range(NQH)]
                k_ps = pj_ps.tile([128, SC], F32, tag="pj")
                v_ps = pj_ps.tile([128, SC], F32, tag="pj")
                for d in range(ND):
                    if sc == 0:
                        # stagger weight loads between x chunks so the first
                        # matmuls aren't stuck behind the full weight DMA
                        nc.sync.dma_start(wq_t[d][:], wqT[bass.ts(d, 128), :])
                        if d == 2:
                            nc.sync.dma_start(cos_t[:], cosT[:, :])
                        elif d == 3:
                            nc.sync.dma_start(sin_t[:], sinT[:, :])
                    if sc % 2 == 0:
                        # x loaded in [128, 1024] tiles spanning two s-chunks
                        # (2KB DMA lines, half the dma_start count)
                        xp_t[d] = xt_pool.tile([128, 2 * SC], BF16, tag="xt",
                                               name="xt")
                        nc.sync.dma_start(
                            xp_t[d][:], xT[bass.ts(d, 128), sc * SC:(sc + 2) * SC])
                    x_t = xp_t[d][:, (sc % 2) * SC:(sc % 2 + 1) * SC]
                    st = (d == 0)
                    for h in range(NQH):
                        nc.tensor.matmul(
                            q_ps[h][:], wq_t[d][:, bass.ts(h, HD)], x_t,
                            start=st, stop=(d == ND - 1))
                    nc.tensor.matmul(
                        k_ps[:], wk_t[:, bass.ts(d, HD)], x_t,
                        start=st, stop=(d == ND - 1))
                    nc.tensor.matmul(
                        v_ps[:], wv_t[:, bass.ts(d, HD)], x_t,
                        start=st, stop=(d == ND - 1))
                # rope q, k
                for h in range(NQH):
                    t1 = rs_pool.tile([128, SC], BF16, tag="t1")
                    t2 = rs_pool.tile([128, SC], BF16, tag="t2")
                    rope(qT_sb[h][:, ssl], q_ps[h], cos_t[:, ssl], sin_t[:, ssl],
                         t1, t2)
                t1 = rs_pool.tile([128, SC], BF16, tag="t1")
                t2 = rs_pool.tile([128, SC], BF16, tag="t2")
                rope(kT_sb[:, ssl], k_ps, cos_t[:, ssl], sin_t[:, ssl], t1, t2)
                # v: copy psum->sbuf, transpose [128,128] blocks to natural
                vt = rs_pool.tile([128, SC], TRDT, tag="vt")
                nc.scalar.copy(vt[:], v_ps[:])
                for i in range(SC // 128):
                    tp = tr_ps.tile([128, 128], TRDT, tag="tr")
                    nc.tensor.transpose(tp[:], vt[:, bass.ts(i, 128)], ident[:])
                    nc.scalar.copy(v_sb[sc * 4 + i][:], tp[:])

        # ================= Stages B + C interleaved per q-chunk =========
        with ExitStack() as ctxbc:
            avnpool = ctxbc.enter_context(tc.tile_pool(name="avn", bufs=1))
            avn_sb = [avnpool.tile([128, S], BF16, tag=f"avn{h}", name=f"avn{h}")
                      for h in range(NQH)]
            sp_ps = ctxbc.enter_context(
                tc.tile_pool(name="sp_ps", bufs=1 + lookahead, space="PSUM"))
            av_ps = ctxbc.enter_context(
                tc.tile_pool(name="av_ps", bufs=2, space="PSUM"))
            o_ps = ctxbc.enter_context(
                tc.tile_pool(name="o_ps", bufs=o_bufs, space="PSUM"))
            p_pool = ctxbc.enter_context(tc.tile_pool(name="p_sb", bufs=4))
            r_pool = ctxbc.enter_context(tc.tile_pool(name="recip", bufs=2))
            o_pool = ctxbc.enter_context(tc.tile_pool(name="o_sb", bufs=2))

            # wo only needed from stage C(j=0); queued after all stage-A DMAs
            for h in range(NQH):
                nc.sync.dma_start(woT_sb[h][:], woT[bass.ts(h, 128), :])

            d_pool = ctxbc.enter_context(tc.tile_pool(name="dacc", bufs=4))

            for j in range(NSC):          # q chunk [512j, 512j+512)
                nkc = 4 * (j + 1)
                for h in range(NQH):
                    av = av_ps.tile([128, SC], F32, tag="av")
                    if dn_mm:
                        dn = o_ps.tile([1, SC], F32, tag="o", name="dn")
                    else:
                        # two private accumulators: DVE owns dacc_v (kc 0 +
                        # odd), gpsimd owns dacc_g (even kc >= 2) — no
                        # cross-engine serial chain.  j=0 is all-diagonal
                        # (trimmed), so gpsimd can't full-width-init there;
                        # DVE takes everything for j=0.
                        use_g = j >= 1
                        dacc_v = d_pool.tile([128, SC], mybir.dt.float32r,
                                             tag="daccv")
                        dacc_g = None
                        if use_g:
                            dacc_g = d_pool.tile([128, SC], mybir.dt.float32r,
                                                 tag="daccg", name="daccg")
                    sts = [None] * nkc
                    w0s = [0] * nkc
                    ps = [None] * nkc

                    def emit_st(kc):
                        # logits st[k, q] for chunk kc, trimmed to live q cols
                        m = kc - 4 * j
                        w0 = (128 * m if m > 0 else 0) if trim else 0
                        w0s[kc] = w0
                        st = sp_ps.tile([128, SC], F32, tag="st")
                        sts[kc] = st
                        nc.tensor.matmul(
                            st[:, w0:SC], kT_sb[:, bass.ts(kc, 128)],
                            qT_sb[h][:, SC * j + w0:SC * (j + 1)],
                            start=True, stop=True)
                        if m >= 0:   # diagonal block: triangular band mask
                            b0 = 128 * m
                            if trim:
                                nc.vector.tensor_tensor(
                                    st[:, b0:b0 + 128], st[:, b0:b0 + 128],
                                    mext[:, 384:512], AluOpType.add)
                            else:
                                nc.vector.tensor_tensor(
                                    st[:, 0:b0 + 128], st[:, 0:b0 + 128],
                                    mext[:, 384 - b0:512], AluOpType.add)

                    for kc0 in range(min(lookahead, nkc)):
                        emit_st(kc0)
                    for kc in range(nkc):
                        if kc + lookahead < nkc:
                            emit_st(kc + lookahead)
                        st, w0 = sts[kc], w0s[kc]
                        p = p_pool.tile([128, SC], BF16, tag="p")
                        ps[kc] = p
                        nc.scalar.activation(
                            p[:, w0:SC], st[:, w0:SC],
                            mybir.ActivationFunctionType.Exp, scale=SCALE)
                        sts[kc] = None
                        nc.tensor.matmul(
                            av[:, w0:SC], v_sb[kc][:], p[:, w0:SC],
                            start=(kc == 0), stop=(kc == nkc - 1))
                        if dn_mm:
                            nc.tensor.matmul(
                                dn[0:1, w0:SC], ones_col[:], p[:, w0:SC],
                                start=(kc == 0), stop=(kc == nkc - 1))
                        elif kc == 0:
                            nc.vector.tensor_copy(dacc_v[:], p[:])
                        elif kc % 2 == 1 or not use_g:
                            nc.vector.tensor_tensor(
                                dacc_v[:, w0:SC], dacc_v[:, w0:SC],
                                p[:, w0:SC], AluOpType.add)
                        elif kc == 2:
                            nc.gpsimd.tensor_copy(dacc_g[:], p[:])
                        else:
                            nc.gpsimd.tensor_tensor(
                                dacc_g[:, w0:SC], dacc_g[:, w0:SC],
                                p[:, w0:SC], AluOpType.add)
                        ps[kc] = None
                    if not dn_mm:
                        dn = o_ps.tile([1, SC], F32, tag="o", name="dn")
                        nc.tensor.matmul(dn[0:1, :], ones_col_r[:],
                                         dacc_v[:], start=True, stop=not use_g)
                        if use_g:
                            nc.tensor.matmul(dn[0:1, :], ones_col_r[:],
                                             dacc_g[:], start=False, stop=True)
                    rcp = r_pool.tile([1, SC], F32, tag="rcp")
                    if fast_recip:
                        nc.vector.reciprocal_approx_fast(rcp[:], dn[0:1, :])
                    else:
                        with nc.allow_low_precision(reason="recip ~1e-4 ok"):
                            nc.vector.reciprocal(rcp[:], dn[0:1, :])
                    bc = r_pool.tile([128, SC], F32, tag="bc")
                    if gps_bcast:
                        nc.gpsimd.partition_broadcast(bc[:], rcp[0:1, :])
                        bcap = bc[:]
                    else:
                        bcp = o_ps.tile([128, SC], F32, tag="o")
                        nc.tensor.matmul(bcp[:], ones_row[:], rcp[0:1, :],
                                         start=True, stop=True)
                        nc.scalar.copy(bc[:], bcp[:])
                        bcap = bc[:]
                    nc.vector.tensor_tensor(
                        avn_sb[h][:, bass.ts(j, SC)], av[:],
                        bcap, AluOpType.mult)

                # ---- Stage C for this q chunk ----
                if debug_dump:
                    continue
                for t in range(4 * j, 4 * j + 4):
                    o_sb = None if psum_dma else o_pool.tile([128, D], F32, tag="o")
                    for dc in range(D // SC):
                        op = o_ps.tile([128, SC], F32, tag="o")
                        for h in range(NQH):
                            nc.tensor.matmul(
                                op[:], avn_sb[h][:, bass.ts(t, 128)],
                                woT_sb[h][:, bass.ts(dc, SC)],
                                start=(h == 0), stop=(h == NQH - 1))
                        if psum_dma:
                            nc.sync.dma_start(
                                out[bass.ts(t, 128), bass.ts(dc, SC)], op[:])
                        elif dc % 2 == 0:
                            nc.scalar.copy(o_sb[:, bass.ts(dc, SC)], op[:])
                        else:
                            nc.vector.tensor_copy(o_sb[:, bass.ts(dc, SC)], op[:])
                    if not psum_dma:
                        nc.sync.dma_start(out[bass.ts(t, 128), :], o_sb[:])

            if debug_dump:
                dbg_pool = ctxbc.enter_context(tc.tile_pool(name="dbg", bufs=2))

                def dump_bf16(row0, src):
                    t = dbg_pool.tile([128, S], F32, tag="dbg")
                    nc.scalar.copy(t[:], src)
                    nc.sync.dma_start(out[row0:row0 + 128, :], t[:])

                for h in range(NQH):
                    dump_bf16(128 * h, qT_sb[h][:])          # rows 0:512
                dump_bf16(512, kT_sb[:])                     # rows 512:640
                for i in range(4):                           # rows 640:1152
                    t = dbg_pool.tile([128, S], F32, tag="dbg")
                    for k2 in range(4):
                        nc.scalar.copy(t[:, bass.ts(k2, 128)],
                                       v_sb[4 * i + k2][:])
                    nc.sync.dma_start(out[640 + 128 * i:768 + 128 * i, 0:512],
                                      t[:, 0:512])
                for h in range(NQH):                         # rows 1280:1792
                    dump_bf16(1280 + 128 * h, avn_sb[h][:])

    nc.compile()
    return nc


# ======================================================================
# Entry point: full-input kernel with internal 8-core sharding
# ======================================================================

import numpy as np


def _install_axon_hooks():
    """Recreate antenv.axon_hooks (absent in this env) so bass_utils works."""
    import types
    if 'antenv.axon_hooks' in sys.modules:
        return
    try:
        import antenv
    except ImportError:
        return
    mod = types.ModuleType('antenv.axon_hooks')
    _state = {'hook': None}
    mod.set_axon_ntff_profile_hook = lambda h: _state.__setitem__('hook', h)
    mod.get_axon_ntff_profile_hook = lambda: _state['hook']
    sys.modules['antenv.axon_hooks'] = mod
    antenv.axon_hooks = mod


_NC_CACHE = {}


def _get_nc():
    if 'nc' not in _NC_CACHE:
        _NC_CACHE['nc'] = build_kernel()
    return _NC_CACHE['nc']


def kernel(**inputs):
    """LlamaAttention forward on 8 NeuronCores.

    Sharding: core c = (batch b = c // 4, head-group g = c % 4); each core
    computes 4 q-heads (1 kv head) for one batch element and its partial
    output through the corresponding wo columns; partials are summed on host.
    """
    _install_axon_hooks()
    from concourse import bass_utils
    import ml_dtypes

    bf16 = ml_dtypes.bfloat16
    hs = np.asarray(inputs["hidden_states"], np.float32)
    cos = np.asarray(inputs["cos"], np.float32)
    sin = np.asarray(inputs["sin"], np.float32)
    wq = np.asarray(inputs["wq"], np.float32)
    wk = np.asarray(inputs["wk"], np.float32)
    wv = np.asarray(inputs["wv"], np.float32)
    wo = np.asarray(inputs["wo"], np.float32)

    in_maps = []
    for c in range(8):
        b, g = c // 4, c % 4
        in_maps.append({
            "xT": np.ascontiguousarray(hs[b].T).astype(bf16),
            "wqT": np.ascontiguousarray(wq[512 * g:512 * (g + 1), :].T).astype(bf16),
            "wkT": np.ascontiguousarray(wk[128 * g:128 * (g + 1), :].T).astype(bf16),
            "wvT": np.ascontiguousarray(wv[128 * g:128 * (g + 1), :].T).astype(bf16),
            "woT": np.ascontiguousarray(wo[:, 512 * g:512 * (g + 1)].T).astype(bf16),
            "cosT": np.ascontiguousarray(cos[b].T).astype(bf16),
            "sinT": np.ascontiguousarray(sin[b].T).astype(bf16),
        })

    nc = _get_nc()
    import os
    _trace = os.environ.get("KERNEL_TRACE", "0") == "1"
    res = bass_utils.run_bass_kernel_spmd(nc, in_maps, core_ids=list(range(8)),
                                          trace=_trace)
    _NC_CACHE['last_result'] = res
    outs = [np.asarray(res.results[c]["out"], np.float32) for c in range(8)]
    full = np.stack([outs[0] + outs[1] + outs[2] + outs[3],
                     outs[4] + outs[5] + outs[6] + outs[7]])
    return full.astype(np.float32)
